# revision 12
# baseline (speedup 1.0000x reference)
"""Trainium2 Bass kernel for nn_ChessMoveSelector (B=4096, NMAX=64).

Reference model:
    board_emb = relu(conv2(relu(conv1(board))).flat @ fc_w.T + fc_b)
                + extra @ extra_w.T + extra_b                      # [B, 256]
    move_emb  = moves @ move_w.T + move_b                          # [B, 64, 128]
    score     = board_emb @ wb.T + move_emb @ wm.T + comb_b        # [B, 64]
    probs     = ragged_softmax_n(score) * (n < lengths)

Key algebraic identity: the softmax runs over n (the move axis), and
board_emb / extra / every bias term contribute a per-row constant that
cancels exactly in the softmax.  The output therefore reduces to

    probs[b, :] = ragged_softmax_n(moves[b, n, :] @ c),  c = move_w.T @ wm

with wm = comb_w[0, 256:].  Only moves, lengths, move_w and comb_w can
affect the output; the conv tower is dead code.  (Verified numerically:
max elementwise relative error vs the full reference is ~2e-5, pure
fp32 rounding.)

Device structure (raw Bacc, manual semaphores, no TileContext):
  * Pure data parallel: B=4096 rows -> 8 cores x 512 rows; each core
    lays rows out as [128 partitions x 4 row-groups], b_local = 4p + t,
    so every partition reads one contiguous 2KB chunk of moves.
  * move_w/wm are replicated across partitions on the host (layout
    only — the sharding hint's "replicate the tiny parameter set") and
    c is computed per-partition on the vector engine.
  * The ragged mask is an additive -1e30 offset folded into the score
    fused-multiply-add; the softmax subtracts the true per-row max
    (reference-exact, robust for any weight draw).
  * The iota constant is generated on-chip by GpSimd (no DMA receipt).
  * Large DMAs are split across the two HWDGE rings (sync + scalar
    engines); the exp runs on the scalar engine.

Measured on 8 axon-tunneled TRN2 NeuronCores: ~19-21 us NEFF exec time
(max across cores), ~7 us of which is the fixed framework preamble.
"""

from contextlib import ExitStack

import numpy as np

import concourse.bass as bass
from concourse import bacc, mybir
from concourse.alu_op_type import AluOpType
from concourse.bass_utils import run_bass_kernel_spmd

N_CORES = 8
B = 4096
NMAX = 64
BD, MD = 256, 128
B_LOCAL = B // N_CORES       # 512
P = 128
T = B_LOCAL // P             # 4

F32 = mybir.dt.float32
I32 = mybir.dt.int32

_CACHE: dict = {}


def _build_program() -> bass.Bass:
    nc = bacc.Bacc("TRN2", target_bir_lowering=False, debug=False)

    moves_d = nc.declare_dram_parameter("moves", [B_LOCAL, NMAX, 2], F32, isOutput=False)
    len_d = nc.declare_dram_parameter("lengths", [B_LOCAL], I32, isOutput=False)
    wrep_d = nc.declare_dram_parameter("wrep", [P, 3, MD], F32, isOutput=False)
    out_d = nc.declare_dram_parameter("out", [B_LOCAL, NMAX], F32, isOutput=True)

    with ExitStack() as ctx:
        en = ctx.enter_context

        mv = en(nc.sbuf_tensor("mv", [P, T, NMAX, 2], F32)).ap()
        len_i = en(nc.sbuf_tensor("len_i", [P, T], I32)).ap()
        iota_i = en(nc.sbuf_tensor("iota_i", [P, T, NMAX], I32)).ap()
        wrep = en(nc.sbuf_tensor("wrep_sb", [P, 3, MD], F32)).ap()
        prod = en(nc.sbuf_tensor("prod", [P, 2, MD], F32)).ap()
        cb = en(nc.sbuf_tensor("cb", [P, 2], F32)).ap()
        moff = en(nc.sbuf_tensor("moff", [P, T, NMAX], F32)).ap()
        rmax = en(nc.sbuf_tensor("rmax", [P, T], F32)).ap()
        smx = en(nc.sbuf_tensor("smx", [P, T, NMAX], F32)).ap()
        minv = en(nc.sbuf_tensor("minv", [P, T, NMAX], F32)).ap()
        s1 = en(nc.sbuf_tensor("s1", [P, T, NMAX], F32)).ap()
        sm = en(nc.sbuf_tensor("sm", [P, T, NMAX], F32)).ap()
        e = en(nc.sbuf_tensor("e", [P, T, NMAX], F32)).ap()
        ssum = en(nc.sbuf_tensor("ssum", [P, T], F32)).ap()
        rec = en(nc.sbuf_tensor("rec", [P, T], F32)).ap()
        outp = en(nc.sbuf_tensor("outp", [P, T, NMAX], F32)).ap()

        d_w = en(nc.semaphore("d_w"))
        s_pl = en(nc.semaphore("s_pl"))
        d_len = en(nc.semaphore("d_len"))
        d_mv = en(nc.semaphore("d_mv"))
        d_out = en(nc.semaphore("d_out"))
        s_dve = en(nc.semaphore("s_dve"))
        s_act = en(nc.semaphore("s_act"))

        with nc.Block() as block:

            HP = P // 2  # split big transfers across the two HWDGE rings
            mv_r = moves_d.ap().rearrange("(p t) n f -> p t n f", p=P)
            out_r = out_d.ap().rearrange("(p t) n -> p t n", p=P)

            @block.scalar
            def _(act: bass.BassEngine):
                act.dma_start(wrep[HP:, :, :], wrep_d.ap()[HP:, :, :]).then_inc(d_w, 16)
                act.dma_start(mv[HP:], mv_r[HP:]).then_inc(d_mv, 16)
                act.activation(
                    e, smx, mybir.ActivationFunctionType.Exp
                )._wait_ge(s_dve, 8).then_inc(s_act, 1)
                act.dma_start(out_r[HP:], outp[HP:])._wait_ge(s_dve, 11).then_inc(
                    d_out, 16
                )

            @block.gpsimd
            def _(pl: bass.BassEngine):
                pl.iota(
                    iota_i, pattern=[[0, T], [1, NMAX]], base=0,
                    channel_multiplier=0,
                ).then_inc(s_pl, 1)

            @block.sync
            def _(sp: bass.BassEngine):
                sp.dma_start(wrep[:HP, :, :], wrep_d.ap()[:HP, :, :]).then_inc(d_w, 16)
                sp.dma_start(mv[:HP], mv_r[:HP]).then_inc(d_mv, 16)
                sp.dma_start(len_i, len_d.ap().rearrange("(p t) -> p t", p=P)).then_inc(
                    d_len, 16
                )
                sp.dma_start(out_r[:HP], outp[:HP])._wait_ge(s_dve, 11).then_inc(
                    d_out, 16
                )
                # final gate: output landed in DRAM before the NEFF ends
                sp.wait_ge(d_out, 32)

            @block.vector
            def _(dve: bass.BassEngine):
                # c[f] = sum_m move_w[m, f] * wm[m], computed on every partition
                dve.wait_ge(d_w, 32)
                dve.tensor_tensor(
                    prod, wrep[:, 0:2, :],
                    wrep[:, 2, :].unsqueeze(1).broadcast_to([P, 2, MD]),
                    op=AluOpType.mult,
                ).then_inc(s_dve, 1)
                dve.tensor_reduce(
                    cb, prod, axis=mybir.AxisListType.X, op=AluOpType.add
                )._wait_ge(s_dve, 1).then_inc(s_dve, 1)
                # additive mask: moff = -1e30 where n >= len, else 0
                dve.wait_ge(s_pl, 1)
                dve.wait_ge(d_len, 16)
                dve.tensor_tensor(
                    minv, iota_i, len_i.unsqueeze(2).broadcast_to([P, T, NMAX]),
                    op=AluOpType.is_ge,
                )._wait_ge(s_dve, 2).then_inc(s_dve, 1)
                dve.tensor_scalar(
                    moff, minv, -1e30, None, op0=AluOpType.mult
                )._wait_ge(s_dve, 3).then_inc(s_dve, 1)
                # masked scores: sm = c0*mv0 + (c1*mv1 + moff)
                dve.wait_ge(d_mv, 32)
                dve.scalar_tensor_tensor(
                    s1, in0=mv[:, :, :, 1], scalar=cb[:, 1:2], in1=moff,
                    op0=AluOpType.mult, op1=AluOpType.add,
                )._wait_ge(s_dve, 4).then_inc(s_dve, 1)
                dve.scalar_tensor_tensor(
                    sm, in0=mv[:, :, :, 0], scalar=cb[:, 0:1], in1=s1,
                    op0=AluOpType.mult, op1=AluOpType.add,
                )._wait_ge(s_dve, 5).then_inc(s_dve, 1)
                # reference-exact ragged softmax: subtract the per-row max
                dve.tensor_reduce(
                    rmax, sm, axis=mybir.AxisListType.X, op=AluOpType.max
                )._wait_ge(s_dve, 6).then_inc(s_dve, 1)
                dve.tensor_tensor(
                    smx, sm, rmax.unsqueeze(2).broadcast_to([P, T, NMAX]),
                    op=AluOpType.subtract,
                )._wait_ge(s_dve, 7).then_inc(s_dve, 1)
                dve.tensor_reduce(
                    ssum, e, axis=mybir.AxisListType.X, op=AluOpType.add
                )._wait_ge(s_act, 1).then_inc(s_dve, 1)
                dve.reciprocal(rec, ssum)._wait_ge(s_dve, 9).then_inc(s_dve, 1)
                dve.tensor_tensor(
                    outp, e, rec.unsqueeze(2).broadcast_to([P, T, NMAX]),
                    op=AluOpType.mult,
                )._wait_ge(s_dve, 10).then_inc(s_dve, 1)

    nc.compile()
    return nc


def _get_program() -> bass.Bass:
    if "nc" not in _CACHE:
        _CACHE["nc"] = _build_program()
    return _CACHE["nc"]


def kernel(**inputs: np.ndarray) -> np.ndarray:
    moves = np.ascontiguousarray(np.asarray(inputs["moves"], dtype=np.float32))
    lengths = np.ascontiguousarray(np.asarray(inputs["lengths"], dtype=np.int32))
    move_w = np.asarray(inputs["move_w"], dtype=np.float32)
    comb_w = np.asarray(inputs["comb_w"], dtype=np.float32)

    # replicate the tiny parameter set across partitions (layout only)
    wrep = np.empty((P, 3, MD), dtype=np.float32)
    wrep[:, 0, :] = move_w[:, 0][None, :]
    wrep[:, 1, :] = move_w[:, 1][None, :]
    wrep[:, 2, :] = comb_w[0, BD:][None, :]

    nc = _get_program()
    in_maps = [
        {
            "moves": moves[i * B_LOCAL : (i + 1) * B_LOCAL],
            "lengths": lengths[i * B_LOCAL : (i + 1) * B_LOCAL],
            "wrep": wrep,
        }
        for i in range(N_CORES)
    ]
    res = run_bass_kernel_spmd(nc, in_maps, core_ids=list(range(N_CORES)))
    return np.concatenate([res.results[i]["out"] for i in range(N_CORES)], axis=0)


# revision 13
# speedup vs baseline: 1.0670x; 1.0670x over previous
"""Trainium2 Bass kernel for nn_ChessMoveSelector (B=4096, NMAX=64).

Reference model:
    board_emb = relu(conv2(relu(conv1(board))).flat @ fc_w.T + fc_b)
                + extra @ extra_w.T + extra_b                      # [B, 256]
    move_emb  = moves @ move_w.T + move_b                          # [B, 64, 128]
    score     = board_emb @ wb.T + move_emb @ wm.T + comb_b        # [B, 64]
    probs     = ragged_softmax_n(score) * (n < lengths)

Key algebraic identity: the softmax runs over n (the move axis), and
board_emb / extra / every bias term contribute a per-row constant that
cancels exactly in the softmax.  The output therefore reduces to

    probs[b, :] = ragged_softmax_n(moves[b, n, :] @ c),  c = move_w.T @ wm

with wm = comb_w[0, 256:].  Only moves, lengths, move_w and comb_w can
affect the output; the conv tower is dead code.  (Verified numerically:
max elementwise relative error vs the full reference is ~2e-5, pure
fp32 rounding.)

Device structure (raw Bacc, manual semaphores, no TileContext):
  * Pure data parallel: B=4096 rows -> 8 cores x 512 rows; each core
    lays rows out as [128 partitions x 4 row-groups], b_local = 4p + t,
    so every partition reads one contiguous 2KB chunk of moves.
  * move_w/wm are replicated across partitions on the host (layout
    only — the sharding hint's "replicate the tiny parameter set") and
    c is computed per-partition on the vector engine.
  * The ragged mask is an additive -1e30 offset folded into the score
    fused-multiply-add; the softmax subtracts the true per-row max
    (reference-exact, robust for any weight draw), fused into 4
    per-row-group scalar-engine exps via bias=-rowmax and accum_out
    (row sums come out of the activation for free).
  * The iota constant is generated on-chip by GpSimd (no DMA receipt).
  * Large DMAs are split across the two HWDGE rings (sync + scalar
    engines); the exp runs on the scalar engine.

Measured on 8 axon-tunneled TRN2 NeuronCores: ~19-21 us NEFF exec time
(max across cores), ~7 us of which is the fixed framework preamble.
"""

from contextlib import ExitStack

import numpy as np

import concourse.bass as bass
from concourse import bacc, mybir
from concourse.alu_op_type import AluOpType
from concourse.bass_utils import run_bass_kernel_spmd

N_CORES = 8
B = 4096
NMAX = 64
BD, MD = 256, 128
B_LOCAL = B // N_CORES       # 512
P = 128
T = B_LOCAL // P             # 4

F32 = mybir.dt.float32
I32 = mybir.dt.int32

_CACHE: dict = {}


def _build_program() -> bass.Bass:
    nc = bacc.Bacc("TRN2", target_bir_lowering=False, debug=False)

    moves_d = nc.declare_dram_parameter("moves", [B_LOCAL, NMAX, 2], F32, isOutput=False)
    len_d = nc.declare_dram_parameter("lengths", [B_LOCAL], I32, isOutput=False)
    wrep_d = nc.declare_dram_parameter("wrep", [P, 3, MD], F32, isOutput=False)
    out_d = nc.declare_dram_parameter("out", [B_LOCAL, NMAX], F32, isOutput=True)

    with ExitStack() as ctx:
        en = ctx.enter_context

        mv = en(nc.sbuf_tensor("mv", [P, T, NMAX, 2], F32)).ap()
        len_i = en(nc.sbuf_tensor("len_i", [P, T], I32)).ap()
        iota_i = en(nc.sbuf_tensor("iota_i", [P, T, NMAX], I32)).ap()
        wrep = en(nc.sbuf_tensor("wrep_sb", [P, 3, MD], F32)).ap()
        prod = en(nc.sbuf_tensor("prod", [P, 2, MD], F32)).ap()
        cb = en(nc.sbuf_tensor("cb", [P, 2], F32)).ap()
        moff = en(nc.sbuf_tensor("moff", [P, T, NMAX], F32)).ap()
        rmaxn = en(nc.sbuf_tensor("rmaxn", [P, T], F32)).ap()
        minv = en(nc.sbuf_tensor("minv", [P, T, NMAX], F32)).ap()
        s1 = en(nc.sbuf_tensor("s1", [P, T, NMAX], F32)).ap()
        sm = en(nc.sbuf_tensor("sm", [P, T, NMAX], F32)).ap()
        e = en(nc.sbuf_tensor("e", [P, T, NMAX], F32)).ap()
        ssum = en(nc.sbuf_tensor("ssum", [P, T], F32)).ap()
        rec = en(nc.sbuf_tensor("rec", [P, T], F32)).ap()
        outp = en(nc.sbuf_tensor("outp", [P, T, NMAX], F32)).ap()

        d_w = en(nc.semaphore("d_w"))
        s_pl = en(nc.semaphore("s_pl"))
        d_len = en(nc.semaphore("d_len"))
        d_mv = en(nc.semaphore("d_mv"))
        d_out = en(nc.semaphore("d_out"))
        s_dve = en(nc.semaphore("s_dve"))
        s_act = en(nc.semaphore("s_act"))

        with nc.Block() as block:

            HP = P // 2  # split big transfers across the two HWDGE rings
            mv_r = moves_d.ap().rearrange("(p t) n f -> p t n f", p=P)
            out_r = out_d.ap().rearrange("(p t) n -> p t n", p=P)

            @block.scalar
            def _(act: bass.BassEngine):
                act.dma_start(wrep[HP:, :, :], wrep_d.ap()[HP:, :, :]).then_inc(d_w, 16)
                act.dma_start(mv[HP:], mv_r[HP:]).then_inc(d_mv, 16)
                # 4 per-group exps: bias = -rowmax, accum_out = row sums
                act.activation(
                    e[:, 0, :], sm[:, 0, :], mybir.ActivationFunctionType.Exp,
                    bias=rmaxn[:, 0:1], accum_out=ssum[:, 0:1],
                )._wait_ge(s_dve, 7).then_inc(s_act, 1)
                for t in range(1, T):
                    act.activation(
                        e[:, t, :], sm[:, t, :], mybir.ActivationFunctionType.Exp,
                        bias=rmaxn[:, t : t + 1], accum_out=ssum[:, t : t + 1],
                    ).then_inc(s_act, 1)
                act.dma_start(out_r[HP:], outp[HP:])._wait_ge(s_dve, 9).then_inc(
                    d_out, 16
                )

            @block.gpsimd
            def _(pl: bass.BassEngine):
                pl.iota(
                    iota_i, pattern=[[0, T], [1, NMAX]], base=0,
                    channel_multiplier=0,
                ).then_inc(s_pl, 1)

            @block.sync
            def _(sp: bass.BassEngine):
                sp.dma_start(wrep[:HP, :, :], wrep_d.ap()[:HP, :, :]).then_inc(d_w, 16)
                sp.dma_start(mv[:HP], mv_r[:HP]).then_inc(d_mv, 16)
                sp.dma_start(len_i, len_d.ap().rearrange("(p t) -> p t", p=P)).then_inc(
                    d_len, 16
                )
                sp.dma_start(out_r[:HP], outp[:HP])._wait_ge(s_dve, 9).then_inc(
                    d_out, 16
                )
                # final gate: output landed in DRAM before the NEFF ends
                sp.wait_ge(d_out, 32)

            @block.vector
            def _(dve: bass.BassEngine):
                # c[f] = sum_m move_w[m, f] * wm[m], computed on every partition
                dve.wait_ge(d_w, 32)
                dve.tensor_tensor(
                    prod, wrep[:, 0:2, :],
                    wrep[:, 2, :].unsqueeze(1).broadcast_to([P, 2, MD]),
                    op=AluOpType.mult,
                ).then_inc(s_dve, 1)
                dve.tensor_reduce(
                    cb, prod, axis=mybir.AxisListType.X, op=AluOpType.add
                )._wait_ge(s_dve, 1).then_inc(s_dve, 1)
                # additive mask: moff = -1e30 where n >= len, else 0
                dve.wait_ge(s_pl, 1)
                dve.wait_ge(d_len, 16)
                dve.tensor_tensor(
                    minv, iota_i, len_i.unsqueeze(2).broadcast_to([P, T, NMAX]),
                    op=AluOpType.is_ge,
                )._wait_ge(s_dve, 2).then_inc(s_dve, 1)
                dve.tensor_scalar(
                    moff, minv, -1e30, None, op0=AluOpType.mult
                )._wait_ge(s_dve, 3).then_inc(s_dve, 1)
                # masked scores: sm = c0*mv0 + (c1*mv1 + moff)
                dve.wait_ge(d_mv, 32)
                dve.scalar_tensor_tensor(
                    s1, in0=mv[:, :, :, 1], scalar=cb[:, 1:2], in1=moff,
                    op0=AluOpType.mult, op1=AluOpType.add,
                )._wait_ge(s_dve, 4).then_inc(s_dve, 1)
                dve.scalar_tensor_tensor(
                    sm, in0=mv[:, :, :, 0], scalar=cb[:, 0:1], in1=s1,
                    op0=AluOpType.mult, op1=AluOpType.add,
                )._wait_ge(s_dve, 5).then_inc(s_dve, 1)
                # reference-exact ragged softmax: negated per-row max feeds
                # the ACT exp as bias; ACT's accum_out produces the row sums
                dve.tensor_reduce(
                    rmaxn, sm, axis=mybir.AxisListType.X, op=AluOpType.max,
                    negate=True,
                )._wait_ge(s_dve, 6).then_inc(s_dve, 1)
                dve.reciprocal(rec, ssum)._wait_ge(s_act, T).then_inc(s_dve, 1)
                dve.tensor_tensor(
                    outp, e, rec.unsqueeze(2).broadcast_to([P, T, NMAX]),
                    op=AluOpType.mult,
                )._wait_ge(s_dve, 8).then_inc(s_dve, 1)

    nc.compile()
    return nc


def _get_program() -> bass.Bass:
    if "nc" not in _CACHE:
        _CACHE["nc"] = _build_program()
    return _CACHE["nc"]


def kernel(**inputs: np.ndarray) -> np.ndarray:
    moves = np.ascontiguousarray(np.asarray(inputs["moves"], dtype=np.float32))
    lengths = np.ascontiguousarray(np.asarray(inputs["lengths"], dtype=np.int32))
    move_w = np.asarray(inputs["move_w"], dtype=np.float32)
    comb_w = np.asarray(inputs["comb_w"], dtype=np.float32)

    # replicate the tiny parameter set across partitions (layout only)
    wrep = np.empty((P, 3, MD), dtype=np.float32)
    wrep[:, 0, :] = move_w[:, 0][None, :]
    wrep[:, 1, :] = move_w[:, 1][None, :]
    wrep[:, 2, :] = comb_w[0, BD:][None, :]

    nc = _get_program()
    in_maps = [
        {
            "moves": moves[i * B_LOCAL : (i + 1) * B_LOCAL],
            "lengths": lengths[i * B_LOCAL : (i + 1) * B_LOCAL],
            "wrep": wrep,
        }
        for i in range(N_CORES)
    ]
    res = run_bass_kernel_spmd(nc, in_maps, core_ids=list(range(N_CORES)))
    return np.concatenate([res.results[i]["out"] for i in range(N_CORES)], axis=0)
